# revision 1
# baseline (speedup 1.0000x reference)
"""Trainium2 Bass kernel for a group-conv / orbit-shared message-passing layer.

Math: out[b, i, o] = sum_{j,c} weight[o, c, pair_orbit[i, j]] * x[b, j, c] + bias[o]

Strategy (pure data parallel over 8 NeuronCores):
  * Host gathers the orbit-shared weight into per-output-position matrices
    W_i[(j,c), o] (24 matrices of 1536x64) and packs them in PAIRS along the
    PE column axis -> 144 stationary tiles of [K=128, M=128] (two i's of 64
    output channels each).
  * Host transposes x to x^T[(j,c), b] so the contraction dim (j,c)=1536 sits
    on SBUF partitions.
  * Each core takes B/8 = 4096 batch columns. For each 512-wide batch block
    and each i-pair, 12 matmuls (K-tiles) accumulate into one PSUM bank:
        psum[(q,o), b] += Wtile[kc, (q,o)].T @ xT[kc, b]
    DVE copies PSUM->SBUF, DMA stores to DRAM.
  * Host reassembles out^T -> (B, 24, 64) and adds bias (bias is zeros in
    practice; cheap host add keeps generality).
"""

import sys

for _p in ("/opt/trn_rl_repo",):
    if _p not in sys.path:
        sys.path.insert(0, _p)

import numpy as np
import ml_dtypes

import concourse.bacc as bacc
import concourse.mybir as mybir
from concourse import tile
from concourse.bass_utils import run_bass_kernel_spmd

B, P, C_IN, C_OUT, N_ORB = 32768, 24, 64, 64, 24
N_CORES = 8
BL = B // N_CORES            # 4096 batch per core
JC = P * C_IN                # 1536 contraction size
KT = JC // 128               # 12 K-tiles
PAIRS = P // 2               # 12 i-pairs
NBB = BL // 512              # 8 batch blocks of 512
NT = PAIRS * KT              # 144 stationary tiles

# "bf16" | "f32r" | "f32"
COMPUTE_DTYPE = "f32r"

_CACHE = {}


def _build(dt_tag):
    if dt_tag == "bf16":
        DT = mybir.dt.bfloat16
    elif dt_tag == "f32r":
        DT = mybir.dt.float32r
    else:
        DT = mybir.dt.float32

    nc = bacc.Bacc(None, target_bir_lowering=False, debug=False)
    xt = nc.dram_tensor("xt", [JC, BL], DT, kind="ExternalInput")
    w = nc.dram_tensor("w", [128, NT * 128], DT, kind="ExternalInput")
    out_t = nc.dram_tensor("out_t", [PAIRS, 128, BL], mybir.dt.float32,
                           kind="ExternalOutput")

    with tile.TileContext(nc) as tc:
        with (
            tc.tile_pool(name="wpool", bufs=1) as wpool,
            tc.tile_pool(name="xpool", bufs=24) as xpool,
            tc.tile_pool(name="opool", bufs=4) as opool,
            tc.tile_pool(name="pspool", bufs=4, space="PSUM") as pspool,
        ):
            # Stationary weights, chunked per i-pair so the first matmuls
            # only wait on a 12-tile chunk, not the whole 4.7-9.4 MB load.
            wchunks = []
            for p in range(PAIRS):
                wc = wpool.tile([128, KT * 128], DT, tag=f"w{p}", name=f"wc{p}")
                nc.sync.dma_start(wc[:], w.ap()[:, p * KT * 128:(p + 1) * KT * 128])
                wchunks.append(wc)

            for bb in range(NBB):
                xk = []
                for k in range(KT):
                    xtile = xpool.tile([128, 512], DT, tag="xk", name=f"x{bb}_{k}")
                    nc.sync.dma_start(
                        xtile[:],
                        xt.ap()[k * 128:(k + 1) * 128, bb * 512:(bb + 1) * 512],
                    )
                    xk.append(xtile)
                for p in range(PAIRS):
                    ps = pspool.tile([128, 512], mybir.dt.float32, tag="ps",
                                     name=f"ps{bb}_{p}")
                    for k in range(KT):
                        nc.tensor.matmul(
                            ps[:],
                            wchunks[p][:, k * 128:(k + 1) * 128],
                            xk[k][:],
                            start=(k == 0),
                            stop=(k == KT - 1),
                        )
                    ob = opool.tile([128, 512], mybir.dt.float32, tag="ob",
                                    name=f"ob{bb}_{p}")
                    nc.vector.tensor_copy(ob[:], ps[:])
                    nc.sync.dma_start(
                        out_t.ap()[p, :, bb * 512:(bb + 1) * 512], ob[:]
                    )

    nc.compile()
    return nc


def _get_nc(dt_tag):
    if dt_tag not in _CACHE:
        _CACHE[dt_tag] = _build(dt_tag)
    return _CACHE[dt_tag]


def _pack_weight(weight, pair_orbit, dt_tag):
    # W_i[(j,c), o] = weight[o, c, pair_orbit[i, j]]
    kern = weight[:, :, np.asarray(pair_orbit)]          # (o, c, i, j)
    wfull = kern.transpose(2, 3, 1, 0).reshape(P, JC, C_OUT)   # (i, jc, o)
    wpair = wfull.reshape(PAIRS, 2, JC, C_OUT)
    # tile t=(p,k): [kc 128, m 128] with m = q*64+o
    wtmp = wpair.transpose(0, 2, 1, 3).reshape(PAIRS, KT, 128, 128)
    wsb = wtmp.reshape(NT, 128, 128).transpose(1, 0, 2).reshape(128, NT * 128)
    if dt_tag == "bf16":
        wsb = wsb.astype(ml_dtypes.bfloat16)
    else:
        wsb = np.ascontiguousarray(wsb, dtype=np.float32)
    return wsb


def _shard_x(x, dt_tag):
    x2 = np.ascontiguousarray(x.reshape(B, JC))
    if dt_tag == "bf16":
        xb = x2.astype(ml_dtypes.bfloat16).view(np.uint16)
        return [
            np.ascontiguousarray(xb[c * BL:(c + 1) * BL].T).view(ml_dtypes.bfloat16)
            for c in range(N_CORES)
        ]
    return [
        np.ascontiguousarray(x2[c * BL:(c + 1) * BL].T) for c in range(N_CORES)
    ]


def kernel(x, weight, bias, pair_orbit):
    x = np.asarray(x, dtype=np.float32)
    weight = np.asarray(weight, dtype=np.float32)
    bias = np.asarray(bias, dtype=np.float32)

    dt_tag = COMPUTE_DTYPE
    nc = _get_nc(dt_tag)

    wsb = _pack_weight(weight, pair_orbit, dt_tag)
    xts = _shard_x(x, dt_tag)
    in_maps = [{"xt": xts[c], "w": wsb} for c in range(N_CORES)]

    res = run_bass_kernel_spmd(nc, in_maps, core_ids=list(range(N_CORES)))

    o = np.stack([res.results[c]["out_t"] for c in range(N_CORES)])
    # (cores, pairs, (q,o), b) -> (b_total, i, o)
    out = (
        o.reshape(N_CORES, PAIRS, 2, C_OUT, BL)
        .transpose(0, 4, 1, 2, 3)
        .reshape(B, P, C_OUT)
    )
    if bias.any():
        out = out + bias
    return np.ascontiguousarray(out)


# revision 3
# speedup vs baseline: 1.0362x; 1.0362x over previous
"""Trainium2 Bass kernel for a group-conv / orbit-shared message-passing layer.

Math: out[b, i, o] = sum_{j,c} weight[o, c, pair_orbit[i, j]] * x[b, j, c] + bias[o]

Strategy (pure data parallel over 8 NeuronCores):
  * Host gathers the orbit-shared weight into per-output-position matrices
    W_i[(j,c), o] (24 matrices of 1536x64), regrouped as moving operands
    Wmov[k, g][kc, (di,o)] of [128, 512] covering 8 output positions each.
  * Host transposes x to x^T[(j,c), b] so the contraction dim (j,c)=1536 sits
    on SBUF partitions; each core takes B/8 = 4096 batch columns.
  * Per 128-batch tile: stationary = x^T k-tile [kc=128, b=128] (one weight
    load per 3 matmuls), moving = Wmov[k, g] [kc=128, 512]; 12 k-tiles
    accumulate into 3 PSUM banks:
        psum_g[b, (di,o)] += xT[kc, b].T @ Wmov[k,g][kc, (di,o)]
    The PSUM free axis (di,o) is already the natural out[b, i, o] layout, so
    stores go straight to a (4096, 24*64) DRAM tensor. No host-side output
    transpose.
  * float32r matmuls: full PE rate at fp32-grade precision (~1.5e-4 rel err).
"""

import sys

for _p in ("/opt/trn_rl_repo",):
    if _p not in sys.path:
        sys.path.insert(0, _p)

import numpy as np
import ml_dtypes

import concourse.bacc as bacc
import concourse.mybir as mybir
from concourse import tile
from concourse.bass_utils import run_bass_kernel_spmd

B, P, C_IN, C_OUT, N_ORB = 32768, 24, 64, 64, 24
N_CORES = 8
BL = B // N_CORES            # 4096 batch per core
JC = P * C_IN                # 1536 contraction size
KT = JC // 128               # 12 K-tiles
NG = 3                       # output groups of 8 positions (8*64 = 512 free)
NBT = BL // 128              # 32 batch tiles per core

# "bf16" | "f32r" | "f32"
COMPUTE_DTYPE = "f32r"

_CACHE = {}


def _build(dt_tag):
    if dt_tag == "bf16":
        DT = mybir.dt.bfloat16
    elif dt_tag == "f32r":
        DT = mybir.dt.float32r
    else:
        DT = mybir.dt.float32

    nc = bacc.Bacc(None, target_bir_lowering=False, debug=False)
    xt = nc.dram_tensor("xt", [JC, BL], DT, kind="ExternalInput")
    w = nc.dram_tensor("w", [128, KT * NG * 512], DT, kind="ExternalInput")
    out_l = nc.dram_tensor("out_l", [BL, P * C_OUT], mybir.dt.float32,
                           kind="ExternalOutput")

    # x^T viewed as [kc=128, k, b] for single-DMA per batch tile
    xt_v = xt.ap().rearrange("(k p) b -> p k b", p=128)

    with tile.TileContext(nc) as tc:
        with (
            tc.tile_pool(name="wpool", bufs=1) as wpool,
            tc.tile_pool(name="xpool", bufs=3) as xpool,
            tc.tile_pool(name="opool", bufs=4) as opool,
            tc.tile_pool(name="pspool", bufs=2, space="PSUM") as pspool,
        ):
            def load_x(bt):
                xbt = xpool.tile([128, KT * 128], DT, tag="xbt", name=f"xb{bt}")
                nc.sync.dma_start(
                    xbt.rearrange("p (k b) -> p k b", k=KT),
                    xt_v[:, :, bt * 128:(bt + 1) * 128],
                )
                return xbt

            # first batch tile's x before the big weight load
            x0 = load_x(0)

            # moving weights, one chunk per k-tile (consumption order)
            wk = []
            for k in range(KT):
                wc = wpool.tile([128, NG * 512], DT, tag=f"w{k}", name=f"wc{k}")
                nc.scalar.dma_start(
                    wc[:], w.ap()[:, k * NG * 512:(k + 1) * NG * 512]
                )
                wk.append(wc)

            xbt = x0
            for bt in range(NBT):
                ps = [
                    pspool.tile([128, 512], mybir.dt.float32, tag=f"ps{g}",
                                name=f"ps{bt}_{g}")
                    for g in range(NG)
                ]
                for k in range(KT):
                    lhsT = xbt[:, k * 128:(k + 1) * 128]
                    for g in range(NG):
                        nc.tensor.matmul(
                            ps[g][:],
                            lhsT,
                            wk[k][:, g * 512:(g + 1) * 512],
                            start=(k == 0),
                            stop=(k == KT - 1),
                        )
                if bt + 1 < NBT:
                    nxt = load_x(bt + 1)
                for g in range(NG):
                    ob = opool.tile([128, 512], mybir.dt.float32, tag="ob",
                                    name=f"ob{bt}_{g}")
                    nc.vector.tensor_copy(ob[:], ps[g][:])
                    nc.sync.dma_start(
                        out_l.ap()[bt * 128:(bt + 1) * 128,
                                   g * 512:(g + 1) * 512],
                        ob[:],
                    )
                if bt + 1 < NBT:
                    xbt = nxt

    nc.compile()
    return nc


def _get_nc(dt_tag):
    if dt_tag not in _CACHE:
        _CACHE[dt_tag] = _build(dt_tag)
    return _CACHE[dt_tag]


def _pack_weight(weight, pair_orbit, dt_tag):
    # W_i[(j,c), o] = weight[o, c, pair_orbit[i, j]]
    kern = weight[:, :, np.asarray(pair_orbit)]          # (o, c, i, j)
    wfull = kern.transpose(2, 3, 1, 0).reshape(P, JC, C_OUT)   # (i, jc, o)
    # Wmov[k, g, kc, di*64+o] = wfull[g*8+di, k*128+kc, o]
    wmov = (
        wfull.reshape(NG, 8, KT, 128, C_OUT)
        .transpose(2, 0, 3, 1, 4)
        .reshape(KT * NG, 128, 512)
    )
    wsb = wmov.transpose(1, 0, 2).reshape(128, KT * NG * 512)
    if dt_tag == "bf16":
        wsb = wsb.astype(ml_dtypes.bfloat16)
    else:
        wsb = np.ascontiguousarray(wsb, dtype=np.float32)
    return wsb


def _shard_x(x, dt_tag):
    x2 = np.ascontiguousarray(x.reshape(B, JC))
    if dt_tag == "bf16":
        xb = x2.astype(ml_dtypes.bfloat16).view(np.uint16)
        return [
            np.ascontiguousarray(xb[c * BL:(c + 1) * BL].T).view(ml_dtypes.bfloat16)
            for c in range(N_CORES)
        ]
    return [
        np.ascontiguousarray(x2[c * BL:(c + 1) * BL].T) for c in range(N_CORES)
    ]


def kernel(x, weight, bias, pair_orbit):
    x = np.asarray(x, dtype=np.float32)
    weight = np.asarray(weight, dtype=np.float32)
    bias = np.asarray(bias, dtype=np.float32)

    dt_tag = COMPUTE_DTYPE
    nc = _get_nc(dt_tag)

    wsb = _pack_weight(weight, pair_orbit, dt_tag)
    xts = _shard_x(x, dt_tag)
    in_maps = [{"xt": xts[c], "w": wsb} for c in range(N_CORES)]

    res = run_bass_kernel_spmd(nc, in_maps, core_ids=list(range(N_CORES)))

    out = np.concatenate(
        [res.results[c]["out_l"] for c in range(N_CORES)], axis=0
    ).reshape(B, P, C_OUT)
    if bias.any():
        out = out + bias
    return out
